# revision 26
# baseline (speedup 1.0000x reference)
"""Trainium2 Bass kernel for nn_BaseModel_88991722373300 (dense_cnn).

ResNet18 stem + layer1 on [64,3,32,32], then dynamic-filter head:
Linear(4096,576)+tanh, L2-normalize over batch dim, per-sample 3x3
dynamic conv, per-sample L2 norm, Linear(64,10).

Distribution (8 NeuronCores, SPMD):
 - stem+layer1 data-parallel over batch (8 samples/core), BN folded into convs
 - AllGather of v=[64,64,8,8] (each core contributes its 8 samples)
 - dynamic-filter linear model-parallel over the 576 outputs (72/core,
   weight slice delivered per-core via in_maps -> no rank-dependent code)
 - batch-dim L2 norm is then local (all 64 samples on-core for its 72 outs)
 - AllToAll redistributes normalized filters back to sample owners
 - per-sample dynamic conv + classifier on-core, each core outputs [8,10]
"""

import sys

for _p in ("/opt/trn_rl_repo",):
    if _p not in sys.path:
        sys.path.insert(0, _p)

import ml_dtypes
import numpy as np

import concourse.bacc as bacc
import concourse.bass as bass
import concourse.mybir as mybir
import concourse.tile as tile
from concourse import bass_utils

F32 = mybir.dt.float32
BF16 = mybir.dt.bfloat16
NPBF = ml_dtypes.bfloat16
AF = mybir.ActivationFunctionType
N_CORES = 8
B = 64            # global batch
SL = 8            # samples per core
EPS = 1e-5        # torch BatchNorm2d eps


# ----------------------------------------------------------------------------
# Host-side preparation (pure numpy layout transforms; no model math beyond
# standard BN constant-folding)
# ----------------------------------------------------------------------------

def _fold_bn(w, bn):
    s, b, m, v = [np.asarray(a, np.float64) for a in bn]
    inv = s / np.sqrt(v + EPS)
    wf = np.asarray(w, np.float64) * inv[:, None, None, None]
    bf = b - m * inv
    return wf.astype(np.float32), bf.astype(np.float32), inv


def _prep_inputs(imgs, conv1_w, bn1, layer1, wdyn_w, wdyn_b, dc_b, wcls_w, wcls_b):
    imgs = np.asarray(imgs, np.float32)
    wdyn_w = np.asarray(wdyn_w, np.float32)
    wdyn_b = np.asarray(wdyn_b, np.float32)
    dc_b = float(np.asarray(dc_b).reshape(-1)[0])
    wcls_w = np.asarray(wcls_w, np.float32)
    wcls_b = np.asarray(wcls_b, np.float32)

    w1f, b1f, inv1 = _fold_bn(conv1_w, bn1)
    assert np.all(inv1 > 0), "maxpool/BN commute requires positive BN scale"

    blk_w, blk_b = [], []
    for blk in layer1:
        (wa, ba), (wb_, bb) = blk
        wfa, bfa, _ = _fold_bn(wa, ba)
        wfb, bfb, _ = _fold_bn(wb_, bb)
        blk_w += [wfa, wfb]
        blk_b += [bfa, bfb]

    # conv1 im2col per core: rows (c,dy,dx) c-major = conv1_w.reshape(64,147)
    # order; cols (sl, y, x), y/x out of 16 (stride 2, pad 3)
    ip = np.pad(imgs, ((0, 0), (0, 0), (3, 3), (3, 3)))
    # windows [B, 3, 7, 7, 16, 16]
    win = np.empty((B, 3, 7, 7, 16, 16), np.float32)
    for dy in range(7):
        for dx in range(7):
            win[:, :, dy, dx] = ip[:, :, dy:dy + 32:2, dx:dx + 32:2]
    # -> [B, 147, 256] -> per-core [147, SL*256]
    col = win.reshape(B, 3 * 49, 256)
    w1T = w1f.reshape(64, 147).T.copy()  # [147, 64]

    # conv3x3 taps blob, tap-pair packed: 6 slots/conv
    # slots 0-2: pairs (0,1),(3,4),(6,7) stacked K=128; slots 3-5: singles 2,5,8
    PAIRS = [(0, 1), (3, 4), (6, 7)]
    SINGLES = [2, 5, 8]
    wb_blob = np.zeros((128, 4 * 6 * 64), np.float32)
    for l in range(4):
        for s_, (ta, tb) in enumerate(PAIRS):
            cs = (l * 6 + s_) * 64
            wb_blob[:64, cs:cs + 64] = blk_w[l][:, :, ta // 3, ta % 3].T
            wb_blob[64:, cs:cs + 64] = blk_w[l][:, :, tb // 3, tb % 3].T
        for s_, t in enumerate(SINGLES):
            cs = (l * 6 + 3 + s_) * 64
            wb_blob[:64, cs:cs + 64] = blk_w[l][:, :, t // 3, t % 3].T

    # consts blob [128, 112]
    consts = np.zeros((128, 112), np.float32)
    consts[:64, 0] = b1f
    for i in range(4):
        consts[:64, 1 + i] = blk_b[i]
    consts[:, 5] = 1.0                        # ones column (all 128)
    consts[:64, 6:16] = wcls_w.T              # wclsT [64px, 10]
    consts[64:, 6:16] = wcls_w.T              # dup for odd-sample tiles
    consts[0, 18:28] = wcls_b                 # wcls_b row
    consts[:, 17] = dc_b                      # dyn-conv shared bias (all 128)
    consts[:72, 32:104] = np.eye(72, dtype=np.float32)  # PE-transpose identity
    # col 16: wdyn_b slice per core (filled per-core below)

    # wdyn slices: core k owns outputs n in [72k, 72k+72); contraction order
    # feature idx = c*64 + px. px-pair folded: lhsT row p=(c + 64*par),
    # col j*72+m -> wdyn_w[72k+m, c*64 + 2j+par]
    w2v = wdyn_w.reshape(576, 64, 64)  # [n, c, px]
    in_maps = []
    for k in range(N_CORES):
        sk = slice(8 * k, 8 * k + 8)
        colk = col[sk].transpose(1, 0, 2).reshape(147, SL * 256)
        wv = w2v[72 * k:72 * k + 72]            # [m, c, px]
        # [p=(par,c), j, m] with p = c + 64*par  -> px = 2j+par
        w2k = np.zeros((128, 32 * 72), np.float32)
        for par in range(2):
            blk = wv[:, :, par::2].transpose(1, 2, 0).reshape(64, 32 * 72)
            w2k[64 * par:64 * par + 64] = blk
        ck = consts.copy()
        ck[:72, 16] = wdyn_b[72 * k:72 * k + 72]
        in_maps.append({
            "x1a": np.ascontiguousarray(colk[:128]).astype(NPBF),
            "x1b": np.ascontiguousarray(colk[128:]).astype(NPBF),
            "w1a": np.ascontiguousarray(w1T[:128]).astype(NPBF),
            "w1b": np.ascontiguousarray(w1T[128:]).astype(NPBF),
            "wb": wb_blob.astype(NPBF),
            "w2": w2k.astype(NPBF),
            "consts": ck,
        })
    return in_maps, dc_b


# ----------------------------------------------------------------------------
# Device program
# ----------------------------------------------------------------------------

def build_program(dc_b: float):
    nc = bacc.Bacc("TRN2", target_bir_lowering=False, debug=False,
                   num_devices=N_CORES)

    x1a_d = nc.dram_tensor("x1a", [128, SL * 256], BF16, kind="ExternalInput")
    x1b_d = nc.dram_tensor("x1b", [19, SL * 256], BF16, kind="ExternalInput")
    w1a_d = nc.dram_tensor("w1a", [128, 64], BF16, kind="ExternalInput")
    w1b_d = nc.dram_tensor("w1b", [19, 64], BF16, kind="ExternalInput")
    wb_d = nc.dram_tensor("wb", [128, 4 * 6 * 64], BF16, kind="ExternalInput")
    w2_d = nc.dram_tensor("w2", [128, 32 * 72], BF16, kind="ExternalInput")
    consts_d = nc.dram_tensor("consts", [128, 112], F32, kind="ExternalInput")
    out_d = nc.dram_tensor("out", [SL, 10], F32, kind="ExternalOutput")

    with tile.TileContext(nc) as tc:
        with tc.tile_pool(name="sb", bufs=1) as sb, \
             tc.tile_pool(name="ps", bufs=2, space="PSUM") as ps, \
             tc.tile_pool(name="ps1", bufs=1, space="PSUM") as ps1, \
             tc.tile_pool(name="dram", bufs=1, space="DRAM") as dram:

            # ------- ncfw wake-up: tiny dummy collective so the real
            # AllGather doesn't pay the ~20us CC-core wake latency -------
            dum_sb = sb.tile([1, 64], BF16, name="dum_sb")
            nc.gpsimd.memset(dum_sb[:], 0.0)
            dum_in = dram.tile([1, 64], BF16, name="dum_in")
            dum_out = dram.tile([N_CORES, 1, 64], BF16, name="dum_out",
                                addr_space="Shared")
            nc.gpsimd.dma_start(dum_in[:], dum_sb[:])
            nc.gpsimd.collective_compute(
                "AllGather", mybir.AluOpType.bypass,
                replica_groups=[list(range(N_CORES))],
                ins=[dum_in[:].opt()], outs=[dum_out[:].opt()])
            # ACT table preloads (Tanh/Sqrt first use costs 1.3us each)
            scr = sb.tile([1, 8], F32, name="scr")
            nc.gpsimd.memset(scr[:], 0.0)
            scr2 = sb.tile([1, 8], F32, name="scr2")
            nc.scalar.activation(scr2[:], scr[:], AF.Sqrt)
            nc.scalar.activation(scr2[:], scr[:], AF.Tanh)

            # ------- input DMAs (issue order ~ consumption order) -------
            w1a = sb.tile([128, 64], BF16, name="w1a_sb")
            w1b = sb.tile([19, 64], BF16, name="w1b_sb")
            consts = sb.tile([128, 112], F32, name="consts_sb")
            x1a = sb.tile([128, SL * 256], BF16, name="x1a_sb")
            x1b = sb.tile([19, SL * 256], BF16, name="x1b_sb")
            wb = sb.tile([128, 4 * 6 * 64], BF16, name="wb_sb")
            w2 = sb.tile([128, 32 * 72], BF16, name="w2_sb")
            nc.sync.dma_start(x1a[:, 0:512], x1a_d[:, 0:512])
            nc.scalar.dma_start(w1a[:], w1a_d[:])
            nc.scalar.dma_start(w1b[:], w1b_d[:])
            nc.sync.dma_start(x1b[:, 0:512], x1b_d[:, 0:512])
            nc.scalar.dma_start(consts[:], consts_d[:])
            for q in range(1, 4):
                cs = slice(512 * q, 512 * (q + 1))
                eng = nc.sync if q % 2 == 0 else nc.scalar
                eng.dma_start(x1a[:, cs], x1a_d[:, cs])
                eng.dma_start(x1b[:, cs], x1b_d[:, cs])
            nc.scalar.dma_start(wb[:], wb_d[:])
            nc.sync.dma_start(w2[:], w2_d[:])

            bias = lambda col: consts[0:64, col:col + 1]
            ones64 = consts[0:64, 5:6]
            wclsT = consts[0:64, 6:16]
            wclsb = consts[0:1, 18:28]
            wdynb = consts[0:72, 16:17]
            eye72 = consts[0:72, 32:104]

            # padded activation tiles [128, (sl, 10, 10)]: rows 64-127 hold
            # the same data shifted by +1 flat position (tap-pair packing)
            pads = {}
            for nm in ("x0p", "y1p", "x1p", "y2p", "vp"):
                np_ = 64 if nm == "vp" else 128
                p = sb.tile([np_, SL * 100], BF16, name=nm)
                nc.gpsimd.memset(p[:], 0.0)
                pads[nm] = p

            def pview(p, lo=True):  # [64, sl, 10, 10]
                base = p[0:64] if lo else p[64:128]
                return base.rearrange("p (s y x) -> p s y x", s=SL, y=10, x=10)

            def interior(p):  # [64, sl, 8, 8] (lower half)
                return pview(p)[:, :, 1:9, 1:9]

            def upper_interior(p):  # upper half, x' = x-1
                return pview(p, lo=False)[:, :, 1:9, 0:8]

            # ------- conv1 (7x7 s2, BN-folded; bias+relu after pool) -------
            # + maxpool 3x3 s2 pad1, commuted before bias/relu
            for q in range(4):          # 2 samples per chunk
                cs = slice(512 * q, 512 * (q + 1))
                p1 = ps1.tile([64, 512], F32, name="p1", tag="conv", bufs=2)
                nc.tensor.matmul(p1[:], w1a[:], x1a[:, cs], start=True, stop=False)
                nc.tensor.matmul(p1[:], w1b[:], x1b[:, cs], start=False, stop=True)
                z = p1[:].rearrange("p (s y x) -> p s y x", s=2, y=16, x=16)
                t1 = sb.tile([64, 2 * 8 * 16], F32, name="t1", tag="t1", bufs=2)
                t1v = t1[:].rearrange("p (s y x) -> p s y x", s=2, y=8, x=16)
                mx = mybir.AluOpType.max
                nc.vector.tensor_copy(t1v, z[:, :, bass.ds(0, 8, 2), :])
                nc.vector.tensor_tensor(t1v, t1v,
                                        z[:, :, bass.ds(1, 8, 2), :], mx)
                nc.vector.tensor_tensor(t1v[:, :, 1:8, :], t1v[:, :, 1:8, :],
                                        z[:, :, bass.ds(1, 7, 2), :], mx)
                p2 = sb.tile([64, 2 * 8 * 8], F32, name="p2", tag="p2", bufs=2)
                p2v = p2[:].rearrange("p (s y x) -> p s y x", s=2, y=8, x=8)
                nc.vector.tensor_tensor(p2v, t1v[:, :, :, bass.ds(0, 8, 2)],
                                        t1v[:, :, :, bass.ds(1, 8, 2)], mx)
                nc.vector.tensor_tensor(p2v[:, :, :, 1:8], p2v[:, :, :, 1:8],
                                        t1v[:, :, :, bass.ds(1, 7, 2)], mx)
                nc.scalar.activation(pview(pads["x0p"])[:, 2 * q:2 * q + 2, 1:9, 1:9],
                                     p2v, AF.Relu, bias=bias(0))
                nc.scalar.activation(
                    pview(pads["x0p"], lo=False)[:, 2 * q:2 * q + 2, 1:9, 0:8],
                    p2v, AF.Relu, bias=bias(0))

            # ------- layer1: 2 BasicBlocks (4 conv3x3, BN folded) -------
            def conv3x3(src_pad, l):
                pc = ps1.tile([64, SL * 64], F32, name="pc", tag="conv", bufs=2)
                sv128 = src_pad[:].rearrange("p (s y x) -> p s y x",
                                             s=SL, y=10, x=10)
                for s_, t in enumerate((0, 3, 6, 2, 5, 8)):
                    dy, dx = t // 3, t % 3
                    K = 128 if s_ < 3 else 64
                    nc.tensor.matmul(
                        pc[:],
                        wb[0:K, (l * 6 + s_) * 64:(l * 6 + s_ + 1) * 64],
                        sv128[0:K, :, dy:dy + 8, dx:dx + 8],
                        start=(s_ == 0), stop=(s_ == 5))
                return pc

            # block1
            pc = conv3x3(pads["x0p"], 0)
            pcv = pc[:].rearrange("p (s y x) -> p s y x", s=SL, y=8, x=8)
            nc.scalar.activation(interior(pads["y1p"]), pcv, AF.Relu, bias=bias(1))
            nc.scalar.activation(upper_interior(pads["y1p"]), pcv, AF.Relu,
                                 bias=bias(1))
            pc = conv3x3(pads["y1p"], 1)
            r1 = sb.tile([64, SL * 64], F32, name="r1")
            nc.vector.tensor_tensor(
                r1[:].rearrange("p (s y x) -> p s y x", s=SL, y=8, x=8),
                pc[:].rearrange("p (s y x) -> p s y x", s=SL, y=8, x=8),
                interior(pads["x0p"]), mybir.AluOpType.add)
            r1v = r1[:].rearrange("p (s y x) -> p s y x", s=SL, y=8, x=8)
            nc.scalar.activation(interior(pads["x1p"]), r1v, AF.Relu, bias=bias(2))
            nc.scalar.activation(upper_interior(pads["x1p"]), r1v, AF.Relu,
                                 bias=bias(2))
            # block2
            pc = conv3x3(pads["x1p"], 2)
            pcv2 = pc[:].rearrange("p (s y x) -> p s y x", s=SL, y=8, x=8)
            nc.scalar.activation(interior(pads["y2p"]), pcv2, AF.Relu, bias=bias(3))
            nc.scalar.activation(upper_interior(pads["y2p"]), pcv2, AF.Relu,
                                 bias=bias(3))
            pc = conv3x3(pads["y2p"], 3)
            r2 = sb.tile([64, SL * 64], F32, name="r2")
            nc.vector.tensor_tensor(
                r2[:].rearrange("p (s y x) -> p s y x", s=SL, y=8, x=8),
                pc[:].rearrange("p (s y x) -> p s y x", s=SL, y=8, x=8),
                interior(pads["x1p"]), mybir.AluOpType.add)
            # v: relu -> padded tile (for dyn conv) and px-parity-folded flat
            # tile (for AG + linear): vfold[c + 64*par, (sl, j)] = v[c, sl, 2j+par]
            vfold = sb.tile([128, SL * 32], BF16, name="vfold")
            r2p = r2[:].rearrange("p (s px) -> p s px", s=SL, px=64)
            nc.scalar.activation(
                vfold[0:64].rearrange("p (s j) -> p s j", s=SL, j=32),
                r2p[:, :, bass.ds(0, 32, 2)], AF.Relu, bias=bias(4))
            nc.scalar.activation(
                vfold[64:128].rearrange("p (s j) -> p s j", s=SL, j=32),
                r2p[:, :, bass.ds(1, 32, 2)], AF.Relu, bias=bias(4))
            nc.scalar.activation(interior(pads["vp"]),
                                 r2[:].rearrange("p (s y x) -> p s y x", s=SL, y=8, x=8),
                                 AF.Relu, bias=bias(4))
            vb_in = dram.tile([128, SL * 32], BF16, name="vb_in")
            vg = dram.tile([N_CORES, 128, SL * 32], BF16, name="vg",
                           addr_space="Shared")
            nc.sync.dma_start(vb_in[:], vfold[:])
            nc.gpsimd.collective_compute(
                "AllGather", mybir.AluOpType.bypass,
                replica_groups=[list(range(N_CORES))],
                ins=[vb_in[:].opt()], outs=[vg[:].opt()])
            # re-warm Tanh/Sqrt ACT tables inside the collective dead
            # window (conv-phase Relu usage evicts the startup preload);
            # reading vfold pins these after the conv phase
            nc.scalar.activation(scr2[:, 0:8], vfold[0:1, 0:8], AF.Sqrt)
            nc.scalar.activation(scr2[:, 0:8], vfold[0:1, 0:8], AF.Tanh)

            # v_all folded [128 (c,par), (k, sl, j)] = [128, 2048]
            vall = sb.tile([128, B * 32], BF16, name="vall")
            vav = vall[:].rearrange("p (k n) -> p k n", k=N_CORES)
            vgv = vg[:].rearrange("k p n -> p k n")
            nc.sync.dma_start(vav[:, 0:4, :], vgv[:, 0:4, :])
            nc.scalar.dma_start(vav[:, 4:8, :], vgv[:, 4:8, :])

            # ------- dynamic-filter linear (72-output slice, all 64 samples)
            # psL[m, s] += sum_c v[c, s, px] * w2[c, px*72+m]  over px
            pl = ps.tile([72, B], F32, name="pl", tag="pl", bufs=1)
            vv = vall[:].rearrange("p (k s j) -> p k s j", k=N_CORES, s=SL, j=32)
            for j in range(32):
                nc.tensor.matmul(pl[:], w2[:, j * 72:(j + 1) * 72],
                                 vv[:, :, :, j], start=(j == 0), stop=(j == 31))
            wsb = sb.tile([72, B], F32, name="wsb")
            nc.scalar.activation(wsb[:], pl[:], AF.Tanh, bias=wdynb)
            # batch-dim L2 norm (free dim here = samples)
            sq = sb.tile([72, B], F32, name="sq")
            ss = sb.tile([72, 1], F32, name="ss")
            nc.scalar.activation(sq[:], wsb[:], AF.Square, accum_out=ss[:])
            nrm = sb.tile([72, 1], F32, name="nrm")
            nc.scalar.activation(nrm[:], ss[:], AF.Sqrt)
            nc.vector.tensor_scalar_max(nrm[:], nrm[:], 1e-12)
            inv = sb.tile([72, 1], F32, name="inv")
            nc.vector.reciprocal(inv[:], nrm[:])
            nc.vector.tensor_scalar_mul(wsb[:], wsb[:], inv[:])

            # ------- redistribute filters to sample owners (AllToAll) -------
            # wnT [s, r] via PE transpose
            pwt = ps.tile([64, 72], F32, name="pwt", tag="pwt", bufs=1)
            nc.tensor.transpose(pwt[:], wsb[:], eye72)
            wnT = sb.tile([64, 72], BF16, name="wnT")
            nc.vector.tensor_copy(wnT[:], pwt[:])
            a2a_in = dram.tile([B, 72], BF16, name="a2a_in")
            a2a_out = dram.tile([N_CORES, SL, 8, 9], BF16, name="a2a_out")
            nc.sync.dma_start(a2a_in[:], wnT[:])
            nc.gpsimd.collective_compute(
                "AllToAll", mybir.AluOpType.bypass,
                replica_groups=[list(range(N_CORES))],
                ins=[a2a_in[:].opt()], outs=[a2a_out[:].opt()])
            # f_sb [64c, (sl, t)]; partition c = 8i + cl from a2a_out[i, sl, cl, t]
            f_sb = sb.tile([64, SL * 9], BF16, name="f_sb")
            for sl in range(SL):
                eng = nc.sync if sl % 2 == 0 else nc.scalar
                eng.dma_start(
                    f_sb[:, sl * 9:(sl + 1) * 9],
                    a2a_out[:, sl, :, :])

            # ------- per-sample dynamic conv: psD[px, sl] -------
            # stationary matmul operands need one flat free dim: materialize
            # the 9 shifted windows contiguously (overlaps the AllGather)
            vwin = sb.tile([64, SL * 9 * 64], BF16, name="vwin")
            vwv = vwin[:].rearrange("p (s t px) -> p s t px", s=SL, t=9, px=64)
            vpv = pview(pads["vp"])
            for t in range(9):
                dy, dx = t // 3, t % 3
                nc.scalar.activation(
                    vwv[:, :, t, :].rearrange("p s (y x) -> p s y x", y=8, x=8),
                    vpv[:, :, dy:dy + 8, dx:dx + 8], AF.Copy)
            pd = ps.tile([64, SL], F32, name="pd", tag="pd", bufs=1)
            for sl in range(SL):
                for t in range(9):
                    nc.tensor.matmul(pd[:, sl:sl + 1],
                                     vwin[:, (sl * 9 + t) * 64:(sl * 9 + t + 1) * 64],
                                     f_sb[:, sl * 9 + t:sl * 9 + t + 1],
                                     start=(t == 0), stop=(t == 8))
            vh = sb.tile([64, SL], F32, name="vh")
            nc.scalar.activation(vh[:], pd[:], AF.Identity, bias=bias(17))
            sqd = sb.tile([64, SL], F32, name="sqd")
            nc.scalar.activation(sqd[:], vh[:], AF.Square)
            pss = ps.tile([1, SL], F32, name="pss", tag="pss", bufs=1)
            nc.tensor.matmul(pss[:], ones64, sqd[:], start=True, stop=True)
            nrm8 = sb.tile([1, SL], F32, name="nrm8")
            nc.scalar.activation(nrm8[:], pss[:], AF.Sqrt)
            inv8 = sb.tile([1, SL], F32, name="inv8")
            nc.vector.reciprocal(inv8[:], nrm8[:])

            # ------- classifier -------
            pcls = ps.tile([1, SL * 10], F32, name="pcls", tag="pcls", bufs=1)
            for sl in range(SL):
                nc.tensor.matmul(pcls[:, sl * 10:(sl + 1) * 10],
                                 vh[:, sl:sl + 1], wclsT,
                                 start=True, stop=True)
            lg = sb.tile([1, SL * 10], F32, name="lg")
            lgv = lg[:].rearrange("p (s n) -> p s n", s=SL, n=10)
            nc.vector.tensor_tensor(
                lgv, pcls[:].rearrange("p (s n) -> p s n", s=SL, n=10),
                inv8[:].unsqueeze(2).broadcast_to([1, SL, 10]),
                mybir.AluOpType.mult)
            nc.vector.tensor_tensor(
                lgv, lgv, wclsb.unsqueeze(1).broadcast_to([1, SL, 10]),
                mybir.AluOpType.add)
            nc.sync.dma_start(out_d[:], lg[:])

    nc.compile()
    return nc


# ----------------------------------------------------------------------------
# Entry points
# ----------------------------------------------------------------------------

_CACHE = {}


def _get_program(dc_b):
    key = round(float(dc_b), 10)
    if key not in _CACHE:
        _CACHE[key] = build_program(dc_b)
    return _CACHE[key]


def _run(in_maps, dc_b, trace=False):
    nc = _get_program(dc_b)
    res = bass_utils.run_bass_kernel_spmd(
        nc, in_maps, core_ids=list(range(N_CORES)), trace=trace)
    out = np.concatenate([res.results[k]["out"] for k in range(N_CORES)], axis=0)
    return out.astype(np.float32), res


def kernel(**inputs) -> np.ndarray:
    in_maps, dc_b = _prep_inputs(**inputs)
    out, _ = _run(in_maps, dc_b, trace=False)
    return out


def kernel_traced(**inputs):
    """Like kernel() but also returns BassKernelResults with exec_time_ns."""
    in_maps, dc_b = _prep_inputs(**inputs)
    return _run(in_maps, dc_b, trace=True)


def simulate(**inputs):
    """Run under MultiCoreSim (no hardware) and return the output."""
    from concourse.bass_interp import MultiCoreSim
    in_maps, dc_b = _prep_inputs(**inputs)
    nc = _get_program(dc_b)
    sim = MultiCoreSim(nc, num_cores=N_CORES)
    for k in range(N_CORES):
        for name, arr in in_maps[k].items():
            sim.cores[k].tensor(name)[:] = arr
    sim.simulate(check_with_hw=False)
    return np.concatenate(
        [sim.cores[k].mem_tensor("out") for k in range(N_CORES)], axis=0
    ).astype(np.float32)


# revision 27
# speedup vs baseline: 1.1936x; 1.1936x over previous
"""Trainium2 Bass kernel for nn_BaseModel_88991722373300 (dense_cnn).

ResNet18 stem + layer1 on [64,3,32,32], then dynamic-filter head:
Linear(4096,576)+tanh, L2-normalize over batch dim, per-sample 3x3
dynamic conv, per-sample L2 norm, Linear(64,10).

Distribution (8 NeuronCores, SPMD):
 - stem+layer1 data-parallel over batch (8 samples/core), BN folded into convs
 - AllGather of v=[64,64,8,8] (each core contributes its 8 samples)
 - dynamic-filter linear model-parallel over the 576 outputs (72/core,
   weight slice delivered per-core via in_maps -> no rank-dependent code)
 - batch-dim L2 norm is then local (all 64 samples on-core for its 72 outs)
 - AllToAll redistributes normalized filters back to sample owners
 - per-sample dynamic conv + classifier on-core, each core outputs [8,10]
"""

import sys

for _p in ("/opt/trn_rl_repo",):
    if _p not in sys.path:
        sys.path.insert(0, _p)

import ml_dtypes
import numpy as np

import concourse.bacc as bacc
import concourse.bass as bass
import concourse.mybir as mybir
import concourse.tile as tile
from concourse import bass_utils

F32 = mybir.dt.float32
BF16 = mybir.dt.bfloat16
NPBF = ml_dtypes.bfloat16
AF = mybir.ActivationFunctionType
N_CORES = 8
B = 64            # global batch
SL = 8            # samples per core
EPS = 1e-5        # torch BatchNorm2d eps


# ----------------------------------------------------------------------------
# Host-side preparation (pure numpy layout transforms; no model math beyond
# standard BN constant-folding)
# ----------------------------------------------------------------------------

def _fold_bn(w, bn):
    s, b, m, v = [np.asarray(a, np.float64) for a in bn]
    inv = s / np.sqrt(v + EPS)
    wf = np.asarray(w, np.float64) * inv[:, None, None, None]
    bf = b - m * inv
    return wf.astype(np.float32), bf.astype(np.float32), inv


def _prep_inputs(imgs, conv1_w, bn1, layer1, wdyn_w, wdyn_b, dc_b, wcls_w, wcls_b):
    imgs = np.asarray(imgs, np.float32)
    wdyn_w = np.asarray(wdyn_w, np.float32)
    wdyn_b = np.asarray(wdyn_b, np.float32)
    dc_b = float(np.asarray(dc_b).reshape(-1)[0])
    wcls_w = np.asarray(wcls_w, np.float32)
    wcls_b = np.asarray(wcls_b, np.float32)

    w1f, b1f, inv1 = _fold_bn(conv1_w, bn1)
    assert np.all(inv1 > 0), "maxpool/BN commute requires positive BN scale"

    blk_w, blk_b = [], []
    for blk in layer1:
        (wa, ba), (wb_, bb) = blk
        wfa, bfa, _ = _fold_bn(wa, ba)
        wfb, bfb, _ = _fold_bn(wb_, bb)
        blk_w += [wfa, wfb]
        blk_b += [bfa, bfb]

    # conv1 im2col per core: rows (c,dy,dx) c-major = conv1_w.reshape(64,147)
    # order; cols (sl, y, x), y/x out of 16 (stride 2, pad 3)
    ip = np.pad(imgs, ((0, 0), (0, 0), (3, 3), (3, 3)))
    # windows [B, 3, 7, 7, 16, 16]
    win = np.empty((B, 3, 7, 7, 16, 16), np.float32)
    for dy in range(7):
        for dx in range(7):
            win[:, :, dy, dx] = ip[:, :, dy:dy + 32:2, dx:dx + 32:2]
    # -> [B, 147, 256] -> per-core [147, SL*256]
    col = win.reshape(B, 3 * 49, 256)
    w1T = w1f.reshape(64, 147).T.copy()  # [147, 64]

    # conv3x3 taps blob, tap-pair packed: 6 slots/conv
    # slots 0-2: pairs (0,1),(3,4),(6,7) stacked K=128; slots 3-5: singles 2,5,8
    PAIRS = [(0, 1), (3, 4), (6, 7)]
    SINGLES = [2, 5, 8]
    wb_blob = np.zeros((128, 4 * 6 * 64), np.float32)
    for l in range(4):
        for s_, (ta, tb) in enumerate(PAIRS):
            cs = (l * 6 + s_) * 64
            wb_blob[:64, cs:cs + 64] = blk_w[l][:, :, ta // 3, ta % 3].T
            wb_blob[64:, cs:cs + 64] = blk_w[l][:, :, tb // 3, tb % 3].T
        for s_, t in enumerate(SINGLES):
            cs = (l * 6 + 3 + s_) * 64
            wb_blob[:64, cs:cs + 64] = blk_w[l][:, :, t // 3, t % 3].T

    # consts blob [128, 112]
    consts = np.zeros((128, 112), np.float32)
    consts[:64, 0] = b1f
    for i in range(4):
        consts[:64, 1 + i] = blk_b[i]
    consts[:, 5] = 1.0                        # ones column (all 128)
    consts[:64, 6:16] = wcls_w.T              # wclsT [64px, 10]
    consts[64:, 6:16] = wcls_w.T              # dup for odd-sample tiles
    consts[0, 18:28] = wcls_b                 # wcls_b row
    consts[:, 17] = dc_b                      # dyn-conv shared bias (all 128)
    consts[:72, 32:104] = np.eye(72, dtype=np.float32)  # PE-transpose identity
    # col 16: wdyn_b slice per core (filled per-core below)

    # wdyn slices: core k owns outputs n in [72k, 72k+72); contraction order
    # feature idx = c*64 + px. px-pair folded: lhsT row p=(c + 64*par),
    # col j*72+m -> wdyn_w[72k+m, c*64 + 2j+par]
    w2v = wdyn_w.reshape(576, 64, 64)  # [n, c, px]
    in_maps = []
    for k in range(N_CORES):
        sk = slice(8 * k, 8 * k + 8)
        colk = col[sk].transpose(1, 0, 2).reshape(147, SL * 256)
        wv = w2v[72 * k:72 * k + 72]            # [m, c, px]
        # [p=(par,c), j, m] with p = c + 64*par  -> px = 2j+par
        w2k = np.zeros((128, 32 * 72), np.float32)
        for par in range(2):
            blk = wv[:, :, par::2].transpose(1, 2, 0).reshape(64, 32 * 72)
            w2k[64 * par:64 * par + 64] = blk
        ck = consts.copy()
        ck[:72, 16] = wdyn_b[72 * k:72 * k + 72]
        in_maps.append({
            "x1a": np.ascontiguousarray(colk[:128]).astype(NPBF),
            "x1b": np.ascontiguousarray(colk[128:]).astype(NPBF),
            "w1a": np.ascontiguousarray(w1T[:128]).astype(NPBF),
            "w1b": np.ascontiguousarray(w1T[128:]).astype(NPBF),
            "wb": wb_blob.astype(NPBF),
            "w2": w2k.astype(NPBF),
            "consts": ck,
        })
    return in_maps, dc_b


# ----------------------------------------------------------------------------
# Device program
# ----------------------------------------------------------------------------

def build_program(dc_b: float):
    nc = bacc.Bacc("TRN2", target_bir_lowering=False, debug=False,
                   num_devices=N_CORES)

    x1a_d = nc.dram_tensor("x1a", [128, SL * 256], BF16, kind="ExternalInput")
    x1b_d = nc.dram_tensor("x1b", [19, SL * 256], BF16, kind="ExternalInput")
    w1a_d = nc.dram_tensor("w1a", [128, 64], BF16, kind="ExternalInput")
    w1b_d = nc.dram_tensor("w1b", [19, 64], BF16, kind="ExternalInput")
    wb_d = nc.dram_tensor("wb", [128, 4 * 6 * 64], BF16, kind="ExternalInput")
    w2_d = nc.dram_tensor("w2", [128, 32 * 72], BF16, kind="ExternalInput")
    consts_d = nc.dram_tensor("consts", [128, 112], F32, kind="ExternalInput")
    out_d = nc.dram_tensor("out", [SL, 10], F32, kind="ExternalOutput")

    with tile.TileContext(nc) as tc:
        with tc.tile_pool(name="sb", bufs=1) as sb, \
             tc.tile_pool(name="ps", bufs=2, space="PSUM") as ps, \
             tc.tile_pool(name="ps1", bufs=1, space="PSUM") as ps1, \
             tc.tile_pool(name="dram", bufs=1, space="DRAM") as dram:

            # ------- ncfw wake-up: tiny dummy collective so the real
            # AllGather doesn't pay the ~20us CC-core wake latency -------
            dum_sb = sb.tile([1, 64], BF16, name="dum_sb")
            nc.gpsimd.memset(dum_sb[:], 0.0)
            dum_in = dram.tile([1, 64], BF16, name="dum_in")
            dum_out = dram.tile([N_CORES, 1, 64], BF16, name="dum_out",
                                addr_space="Shared")
            nc.gpsimd.dma_start(dum_in[:], dum_sb[:])
            nc.gpsimd.collective_compute(
                "AllGather", mybir.AluOpType.bypass,
                replica_groups=[list(range(N_CORES))],
                ins=[dum_in[:].opt()], outs=[dum_out[:].opt()])
            # ACT table preloads (Tanh/Sqrt first use costs 1.3us each)
            scr = sb.tile([1, 8], F32, name="scr")
            nc.gpsimd.memset(scr[:], 0.0)
            scr2 = sb.tile([1, 8], F32, name="scr2")
            nc.scalar.activation(scr2[:], scr[:], AF.Sqrt)
            nc.scalar.activation(scr2[:], scr[:], AF.Tanh)

            # ------- input DMAs (issue order ~ consumption order) -------
            w1a = sb.tile([128, 64], BF16, name="w1a_sb")
            w1b = sb.tile([19, 64], BF16, name="w1b_sb")
            consts = sb.tile([128, 112], F32, name="consts_sb")
            x1a = sb.tile([128, SL * 256], BF16, name="x1a_sb")
            x1b = sb.tile([19, SL * 256], BF16, name="x1b_sb")
            wb = sb.tile([128, 4 * 6 * 64], BF16, name="wb_sb")
            w2 = sb.tile([128, 32 * 72], BF16, name="w2_sb")
            nc.sync.dma_start(x1a[:, 0:512], x1a_d[:, 0:512])
            nc.scalar.dma_start(w1a[:], w1a_d[:])
            nc.scalar.dma_start(w1b[:], w1b_d[:])
            nc.sync.dma_start(x1b[:, 0:512], x1b_d[:, 0:512])
            nc.scalar.dma_start(consts[:], consts_d[:])
            for q in range(1, 4):
                cs = slice(512 * q, 512 * (q + 1))
                eng = nc.sync if q % 2 == 0 else nc.scalar
                eng.dma_start(x1a[:, cs], x1a_d[:, cs])
                eng.dma_start(x1b[:, cs], x1b_d[:, cs])
            nc.scalar.dma_start(wb[:], wb_d[:])
            nc.sync.dma_start(w2[:], w2_d[:])

            bias = lambda col: consts[0:64, col:col + 1]
            ones64 = consts[0:64, 5:6]
            wclsT = consts[0:64, 6:16]
            wclsb = consts[0:1, 18:28]
            wdynb = consts[0:72, 16:17]
            eye72 = consts[0:72, 32:104]

            # padded activation tiles [128, (sl, 10, 10)]: rows 64-127 hold
            # the same data shifted by +1 flat position (tap-pair packing)
            pads = {}
            for nm in ("x0p", "y1p", "x1p", "y2p", "vp"):
                np_ = 64 if nm == "vp" else 128
                p = sb.tile([np_, SL * 100], BF16, name=nm)
                nc.gpsimd.memset(p[:], 0.0)
                pads[nm] = p

            def pview(p, lo=True):  # [64, sl, 10, 10]
                base = p[0:64] if lo else p[64:128]
                return base.rearrange("p (s y x) -> p s y x", s=SL, y=10, x=10)

            def interior(p):  # [64, sl, 8, 8] (lower half)
                return pview(p)[:, :, 1:9, 1:9]

            def upper_interior(p):  # upper half, x' = x-1
                return pview(p, lo=False)[:, :, 1:9, 0:8]

            # ------- conv1 (7x7 s2, BN-folded; bias+relu after pool) -------
            # + maxpool 3x3 s2 pad1, commuted before bias/relu
            chunks = [(0, 2), (2, 2), (4, 2), (6, 1), (7, 1)]
            for s0, ns in chunks:
                cs = slice(256 * s0, 256 * (s0 + ns))
                p1 = ps1.tile([64, 512], F32, name="p1", tag="conv", bufs=2)
                p1v = p1[:, 0:256 * ns]
                nc.tensor.matmul(p1v, w1a[:], x1a[:, cs], start=True, stop=False)
                nc.tensor.matmul(p1v, w1b[:], x1b[:, cs], start=False, stop=True)
                z = p1v.rearrange("p (s y x) -> p s y x", s=ns, y=16, x=16)
                t1 = sb.tile([64, 2 * 8 * 16], F32, name="t1", tag="t1", bufs=2)
                t1v = t1[:, 0:128 * ns].rearrange("p (s y x) -> p s y x",
                                                  s=ns, y=8, x=16)
                mx = mybir.AluOpType.max
                nc.vector.tensor_copy(t1v, z[:, :, bass.ds(0, 8, 2), :])
                nc.vector.tensor_tensor(t1v, t1v,
                                        z[:, :, bass.ds(1, 8, 2), :], mx)
                nc.vector.tensor_tensor(t1v[:, :, 1:8, :], t1v[:, :, 1:8, :],
                                        z[:, :, bass.ds(1, 7, 2), :], mx)
                p2 = sb.tile([64, 2 * 8 * 8], F32, name="p2", tag="p2", bufs=2)
                p2v = p2[:, 0:64 * ns].rearrange("p (s y x) -> p s y x",
                                                 s=ns, y=8, x=8)
                nc.vector.tensor_tensor(p2v, t1v[:, :, :, bass.ds(0, 8, 2)],
                                        t1v[:, :, :, bass.ds(1, 8, 2)], mx)
                nc.vector.tensor_tensor(p2v[:, :, :, 1:8], p2v[:, :, :, 1:8],
                                        t1v[:, :, :, bass.ds(1, 7, 2)], mx)
                nc.scalar.activation(pview(pads["x0p"])[:, s0:s0 + ns, 1:9, 1:9],
                                     p2v, AF.Relu, bias=bias(0))
                nc.scalar.activation(
                    pview(pads["x0p"], lo=False)[:, s0:s0 + ns, 1:9, 0:8],
                    p2v, AF.Relu, bias=bias(0))

            # ------- layer1: 2 BasicBlocks (4 conv3x3, BN folded) -------
            def conv3x3(src_pad, l):
                pc = ps1.tile([64, SL * 64], F32, name="pc", tag="conv", bufs=2)
                sv128 = src_pad[:].rearrange("p (s y x) -> p s y x",
                                             s=SL, y=10, x=10)
                for s_, t in enumerate((0, 3, 6, 2, 5, 8)):
                    dy, dx = t // 3, t % 3
                    K = 128 if s_ < 3 else 64
                    nc.tensor.matmul(
                        pc[:],
                        wb[0:K, (l * 6 + s_) * 64:(l * 6 + s_ + 1) * 64],
                        sv128[0:K, :, dy:dy + 8, dx:dx + 8],
                        start=(s_ == 0), stop=(s_ == 5))
                return pc

            # block1
            pc = conv3x3(pads["x0p"], 0)
            pcv = pc[:].rearrange("p (s y x) -> p s y x", s=SL, y=8, x=8)
            nc.scalar.activation(interior(pads["y1p"]), pcv, AF.Relu, bias=bias(1))
            nc.scalar.activation(upper_interior(pads["y1p"]), pcv, AF.Relu,
                                 bias=bias(1))
            pc = conv3x3(pads["y1p"], 1)
            r1 = sb.tile([64, SL * 64], F32, name="r1")
            nc.vector.tensor_tensor(
                r1[:].rearrange("p (s y x) -> p s y x", s=SL, y=8, x=8),
                pc[:].rearrange("p (s y x) -> p s y x", s=SL, y=8, x=8),
                interior(pads["x0p"]), mybir.AluOpType.add)
            r1v = r1[:].rearrange("p (s y x) -> p s y x", s=SL, y=8, x=8)
            nc.scalar.activation(interior(pads["x1p"]), r1v, AF.Relu, bias=bias(2))
            nc.scalar.activation(upper_interior(pads["x1p"]), r1v, AF.Relu,
                                 bias=bias(2))
            # block2
            pc = conv3x3(pads["x1p"], 2)
            pcv2 = pc[:].rearrange("p (s y x) -> p s y x", s=SL, y=8, x=8)
            nc.scalar.activation(interior(pads["y2p"]), pcv2, AF.Relu, bias=bias(3))
            nc.scalar.activation(upper_interior(pads["y2p"]), pcv2, AF.Relu,
                                 bias=bias(3))
            pc = conv3x3(pads["y2p"], 3)
            r2 = sb.tile([64, SL * 64], F32, name="r2")
            nc.vector.tensor_tensor(
                r2[:].rearrange("p (s y x) -> p s y x", s=SL, y=8, x=8),
                pc[:].rearrange("p (s y x) -> p s y x", s=SL, y=8, x=8),
                interior(pads["x1p"]), mybir.AluOpType.add)
            # v: relu -> padded tile (for dyn conv) and px-parity-folded flat
            # tile (for AG + linear): vfold[c + 64*par, (sl, j)] = v[c, sl, 2j+par]
            vfold = sb.tile([128, SL * 32], BF16, name="vfold")
            r2p = r2[:].rearrange("p (s px) -> p s px", s=SL, px=64)
            nc.scalar.activation(
                vfold[0:64].rearrange("p (s j) -> p s j", s=SL, j=32),
                r2p[:, :, bass.ds(0, 32, 2)], AF.Relu, bias=bias(4))
            nc.scalar.activation(
                vfold[64:128].rearrange("p (s j) -> p s j", s=SL, j=32),
                r2p[:, :, bass.ds(1, 32, 2)], AF.Relu, bias=bias(4))
            nc.scalar.activation(interior(pads["vp"]),
                                 r2[:].rearrange("p (s y x) -> p s y x", s=SL, y=8, x=8),
                                 AF.Relu, bias=bias(4))
            vb_in = dram.tile([128, SL * 32], BF16, name="vb_in")
            vg = dram.tile([N_CORES, 128, SL * 32], BF16, name="vg",
                           addr_space="Shared")
            nc.sync.dma_start(vb_in[:], vfold[:])
            nc.gpsimd.collective_compute(
                "AllGather", mybir.AluOpType.bypass,
                replica_groups=[list(range(N_CORES))],
                ins=[vb_in[:].opt()], outs=[vg[:].opt()])
            # re-warm Tanh table in the collective dead window (conv-phase
            # Relu usage evicts the startup preload); pinned by reading vfold
            nc.scalar.activation(scr2[:, 0:8], vfold[0:1, 0:8], AF.Tanh)

            # v_all folded [128 (c,par), (k, sl, j)] = [128, 2048]
            vall = sb.tile([128, B * 32], BF16, name="vall")
            vav = vall[:].rearrange("p (k n) -> p k n", k=N_CORES)
            vgv = vg[:].rearrange("k p n -> p k n")
            nc.sync.dma_start(vav[:, 0:4, :], vgv[:, 0:4, :])
            nc.scalar.dma_start(vav[:, 4:8, :], vgv[:, 4:8, :])

            # ------- dynamic-filter linear (72-output slice, all 64 samples)
            # psL[m, s] += sum_c v[c, s, px] * w2[c, px*72+m]  over px
            # Sqrt table prewarm during the linear (Tanh load evicts Sqrt);
            # pinned by reading vall
            nc.scalar.activation(scr2[:, 0:8], vall[0:1, 0:8], AF.Sqrt)
            pl = ps.tile([72, B], F32, name="pl", tag="pl", bufs=1)
            vv = vall[:].rearrange("p (k s j) -> p k s j", k=N_CORES, s=SL, j=32)
            for j in range(32):
                nc.tensor.matmul(pl[:], w2[:, j * 72:(j + 1) * 72],
                                 vv[:, :, :, j], start=(j == 0), stop=(j == 31))
            wsb = sb.tile([72, B], F32, name="wsb")
            nc.scalar.activation(wsb[:], pl[:], AF.Tanh, bias=wdynb)
            # batch-dim L2 norm (free dim here = samples)
            sq = sb.tile([72, B], F32, name="sq")
            ss = sb.tile([72, 1], F32, name="ss")
            nc.scalar.activation(sq[:], wsb[:], AF.Square, accum_out=ss[:])
            nrm = sb.tile([72, 1], F32, name="nrm")
            nc.scalar.activation(nrm[:], ss[:], AF.Sqrt)
            nc.vector.tensor_scalar_max(nrm[:], nrm[:], 1e-12)
            inv = sb.tile([72, 1], F32, name="inv")
            nc.vector.reciprocal(inv[:], nrm[:])
            nc.vector.tensor_scalar_mul(wsb[:], wsb[:], inv[:])

            # ------- redistribute filters to sample owners (AllToAll) -------
            # wnT [s, r] via PE transpose
            pwt = ps.tile([64, 72], F32, name="pwt", tag="pwt", bufs=1)
            nc.tensor.transpose(pwt[:], wsb[:], eye72)
            wnT = sb.tile([64, 72], BF16, name="wnT")
            nc.vector.tensor_copy(wnT[:], pwt[:])
            a2a_in = dram.tile([B, 72], BF16, name="a2a_in")
            a2a_out = dram.tile([N_CORES, SL, 8, 9], BF16, name="a2a_out")
            nc.sync.dma_start(a2a_in[:], wnT[:])
            nc.gpsimd.collective_compute(
                "AllToAll", mybir.AluOpType.bypass,
                replica_groups=[list(range(N_CORES))],
                ins=[a2a_in[:].opt()], outs=[a2a_out[:].opt()])
            # f_sb [64c, (sl, t)]; partition c = 8i + cl from a2a_out[i, sl, cl, t]
            f_sb = sb.tile([64, SL * 9], BF16, name="f_sb")
            for sl in range(SL):
                eng = nc.sync if sl % 2 == 0 else nc.scalar
                eng.dma_start(
                    f_sb[:, sl * 9:(sl + 1) * 9],
                    a2a_out[:, sl, :, :])

            # ------- per-sample dynamic conv: psD[px, sl] -------
            # stationary matmul operands need one flat free dim: materialize
            # the 9 shifted windows contiguously (overlaps the AllGather)
            vwin = sb.tile([64, SL * 9 * 64], BF16, name="vwin")
            vwv = vwin[:].rearrange("p (s t px) -> p s t px", s=SL, t=9, px=64)
            vpv = pview(pads["vp"])
            for t in range(9):
                dy, dx = t // 3, t % 3
                nc.scalar.activation(
                    vwv[:, :, t, :].rearrange("p s (y x) -> p s y x", y=8, x=8),
                    vpv[:, :, dy:dy + 8, dx:dx + 8], AF.Copy)
            pd = ps.tile([64, SL], F32, name="pd", tag="pd", bufs=1)
            for sl in range(SL):
                for t in range(9):
                    nc.tensor.matmul(pd[:, sl:sl + 1],
                                     vwin[:, (sl * 9 + t) * 64:(sl * 9 + t + 1) * 64],
                                     f_sb[:, sl * 9 + t:sl * 9 + t + 1],
                                     start=(t == 0), stop=(t == 8))
            vh = sb.tile([64, SL], F32, name="vh")
            nc.scalar.activation(vh[:], pd[:], AF.Identity, bias=bias(17))
            sqd = sb.tile([64, SL], F32, name="sqd")
            nc.scalar.activation(sqd[:], vh[:], AF.Square)
            pss = ps.tile([1, SL], F32, name="pss", tag="pss", bufs=1)
            nc.tensor.matmul(pss[:], ones64, sqd[:], start=True, stop=True)
            nrm8 = sb.tile([1, SL], F32, name="nrm8")
            nc.scalar.activation(nrm8[:], pss[:], AF.Sqrt)
            inv8 = sb.tile([1, SL], F32, name="inv8")
            nc.vector.reciprocal(inv8[:], nrm8[:])

            # ------- classifier -------
            pcls = ps.tile([1, SL * 10], F32, name="pcls", tag="pcls", bufs=1)
            for sl in range(SL):
                nc.tensor.matmul(pcls[:, sl * 10:(sl + 1) * 10],
                                 vh[:, sl:sl + 1], wclsT,
                                 start=True, stop=True)
            lg = sb.tile([1, SL * 10], F32, name="lg")
            lgv = lg[:].rearrange("p (s n) -> p s n", s=SL, n=10)
            nc.vector.tensor_tensor(
                lgv, pcls[:].rearrange("p (s n) -> p s n", s=SL, n=10),
                inv8[:].unsqueeze(2).broadcast_to([1, SL, 10]),
                mybir.AluOpType.mult)
            nc.vector.tensor_tensor(
                lgv, lgv, wclsb.unsqueeze(1).broadcast_to([1, SL, 10]),
                mybir.AluOpType.add)
            nc.sync.dma_start(out_d[:], lg[:])

    nc.compile()
    return nc


# ----------------------------------------------------------------------------
# Entry points
# ----------------------------------------------------------------------------

_CACHE = {}


def _get_program(dc_b):
    key = round(float(dc_b), 10)
    if key not in _CACHE:
        _CACHE[key] = build_program(dc_b)
    return _CACHE[key]


def _run(in_maps, dc_b, trace=False):
    nc = _get_program(dc_b)
    res = bass_utils.run_bass_kernel_spmd(
        nc, in_maps, core_ids=list(range(N_CORES)), trace=trace)
    out = np.concatenate([res.results[k]["out"] for k in range(N_CORES)], axis=0)
    return out.astype(np.float32), res


def kernel(**inputs) -> np.ndarray:
    in_maps, dc_b = _prep_inputs(**inputs)
    out, _ = _run(in_maps, dc_b, trace=False)
    return out


def kernel_traced(**inputs):
    """Like kernel() but also returns BassKernelResults with exec_time_ns."""
    in_maps, dc_b = _prep_inputs(**inputs)
    return _run(in_maps, dc_b, trace=True)


def simulate(**inputs):
    """Run under MultiCoreSim (no hardware) and return the output."""
    from concourse.bass_interp import MultiCoreSim
    in_maps, dc_b = _prep_inputs(**inputs)
    nc = _get_program(dc_b)
    sim = MultiCoreSim(nc, num_cores=N_CORES)
    for k in range(N_CORES):
        for name, arr in in_maps[k].items():
            sim.cores[k].tensor(name)[:] = arr
    sim.simulate(check_with_hw=False)
    return np.concatenate(
        [sim.cores[k].mem_tensor("out") for k in range(N_CORES)], axis=0
    ).astype(np.float32)
